# revision 2
# baseline (speedup 1.0000x reference)
"""TRN2 Bass kernel for nn_CustomHeadMultiHeadAttention (dense transformer).

Full inputs: x [8, 2048, 1024] f32 + QKV/classify weights. Sharding: pure
data parallelism — batch 8 across 8 NeuronCores, one batch element per core.
Each core runs the complete MHA + GELU + classify on its slice; no
collectives. Host only slices the batch and stacks/transposes the outputs.

Per-core pipeline (bf16 matmul operands, fp32 PSUM accumulation):
  xT    = PE-transpose(x)                  [h, s] layout
  qT/kT = Wq/Wk (lhsT) @ xT + b            [d, s]; d-tile == head
  v     = xT-chunks (lhsT) @ Wv + bv       [s, d] natural
  per (head, 512-wide q block):
    scores^T tiles = kT-chunk (lhsT) @ qT  [k=128, q=512] (single MM each,
                                            contraction = d_k = 128)
    P^T  = exp(scores^T / sqrt(dk))        ACT drains PSUM->SBUF bf16
                                            (no max-subtraction: scores~N(0,1))
    dn   = sum_kt P^T[kt]                  DVE chain (f32 partial sums)
    attn^T = sum_kt v-chunk (lhsT) @ P^T[kt]   [d=128, q=512]
    denom broadcast = ones[128,128] @ dn   one f32r matmul
    an   = attn^T * reciprocal(denom)
  h^T   = gelu(an) in place; logits^T = Wc-chunks (lhsT) @ h^T + bc  [2, s]
Host transposes logits^T -> [2048, 2].
"""

import math
import sys

sys.path.insert(0, "/opt/trn_rl_repo")

import numpy as np

import concourse.bass as bass
import concourse.mybir as mybir
import concourse.tile as tile
from concourse import bacc
from concourse.bass_utils import run_bass_kernel_spmd
from concourse.masks import make_identity

AF = mybir.ActivationFunctionType
ALU = mybir.AluOpType
F32 = mybir.dt.float32
F32R = mybir.dt.float32r
BF16 = mybir.dt.bfloat16

B = 8           # batch (== number of cores)
S = 2048        # sequence length
H = 1024        # hidden
NH = 8          # heads
DK = 128        # head dim
P = 128         # partitions
NC = 2          # classes
SB = S // 512   # 4 q/s blocks of 512
HT = H // P     # 8 hidden tiles
ST = S // P     # 16 seq tiles
SCALE = 1.0 / math.sqrt(DK)

_NC_CACHE = []


def _build():
    nc = bacc.Bacc(None, target_bir_lowering=False, debug=False)

    x = nc.dram_tensor("x", [S, H], F32, kind="ExternalInput")
    Wq = nc.dram_tensor("Wq", [H, H], F32, kind="ExternalInput")
    bq = nc.dram_tensor("bq", [H], F32, kind="ExternalInput")
    Wk = nc.dram_tensor("Wk", [H, H], F32, kind="ExternalInput")
    bk = nc.dram_tensor("bk", [H], F32, kind="ExternalInput")
    Wv = nc.dram_tensor("Wv", [H, H], F32, kind="ExternalInput")
    bv = nc.dram_tensor("bv", [H], F32, kind="ExternalInput")
    Wc = nc.dram_tensor("Wc", [H, NC], F32, kind="ExternalInput")
    bc = nc.dram_tensor("bc", [NC], F32, kind="ExternalInput")
    out = nc.dram_tensor("out", [NC, S], F32, kind="ExternalOutput")

    with tile.TileContext(nc) as tc:
        with (
            tc.tile_pool(name="persist", bufs=1) as persist,
            tc.tile_pool(name="late", bufs=1) as late,
        ):
            ident = persist.tile([P, P], BF16, tag="ident")
            make_identity(nc, ident)
            ones128 = persist.tile([P, P], BF16, tag="ones128")
            nc.vector.memset(ones128, 1.0)
            bq_sb = persist.tile([P, HT], F32, tag="bq")
            bk_sb = persist.tile([P, HT], F32, tag="bk")
            nc.sync.dma_start(bq_sb, bq.rearrange("(j p) -> p j", p=P))
            nc.sync.dma_start(bk_sb, bk.rearrange("(j p) -> p j", p=P))
            bv_bc = persist.tile([P, H], BF16, tag="bv")
            nc.gpsimd.dma_start(bv_bc, bv[None, :].to_broadcast((P, H)))
            bc_sb = persist.tile([NC, 1], F32, tag="bc")
            nc.sync.dma_start(bc_sb, bc[:, None])
            wc_sb = persist.tile([P, HT, NC], BF16, tag="wc")
            nc.gpsimd.dma_start(wc_sb, Wc.rearrange("(j p) c -> p j c", p=P))

            qT = late.tile([P, HT, S], BF16, tag="qT")
            kT = late.tile([P, HT, S], BF16, tag="kT")
            v_sb = late.tile([P, ST, H], BF16, tag="v")
            an = late.tile([P, HT, S], BF16, tag="an")
            louT = late.tile([NC, SB, 512], F32, tag="louT")

            # ---- Phase A/B/C: load + transpose x, stream weights, QKV ----
            with tc.tile_pool(name="early", bufs=1) as early:
                xT = early.tile([P, HT, S], BF16, tag="xT")
                wv_sb = early.tile([P, HT, H], BF16, tag="wv")
                nc.gpsimd.dma_start(wv_sb, Wv.rearrange("(o p) d -> p o d", p=P))
                wq_r = Wq.rearrange("(o p) d -> p o d", p=P)
                wk_r = Wk.rearrange("(o p) d -> p o d", p=P)

                with (
                    tc.tile_pool(name="xload", bufs=3) as xload,
                    tc.tile_pool(name="tps", bufs=2, space="PSUM") as tpsum,
                ):
                    for st in range(ST):
                        xt = xload.tile([P, H], BF16, tag="xl")
                        nc.gpsimd.dma_start(xt, x[st * P:(st + 1) * P, :])
                        for jg in range(2):
                            ps = tpsum.tile([P, 4, P], BF16, tag="tp")
                            for j4 in range(4):
                                j = jg * 4 + j4
                                nc.tensor.transpose(
                                    ps[:, j4, :], xt[:, j * P:(j + 1) * P], ident
                                )
                            nc.vector.tensor_copy(
                                xT[:, jg * 4:(jg + 1) * 4, st * P:(st + 1) * P],
                                ps[:],
                            )

                with (
                    tc.tile_pool(name="qkvps", bufs=4, space="PSUM") as qkvps,
                    tc.tile_pool(name="wstream", bufs=3) as wstream,
                ):
                    for w_r, b_sb, oT in ((wq_r, bq_sb, qT), (wk_r, bk_sb, kT)):
                        for j in range(HT):
                            wj = wstream.tile([P, HT, P], BF16, tag="wj")
                            nc.gpsimd.dma_start(wj, w_r[:, :, j * P:(j + 1) * P])
                            pss = [
                                qkvps.tile([P, 2, 512], F32, tag="qkv",
                                           name=f"qkv_{sg}")
                                for sg in range(2)
                            ]
                            for hi in range(HT):
                                lhsT = wj[:, hi, :]
                                for sg in range(2):
                                    for sh in range(2):
                                        nc.tensor.matmul(
                                            pss[sg][:, sh, :],
                                            lhsT,
                                            xT[:, hi,
                                               sg * 1024 + sh * 512:
                                               sg * 1024 + (sh + 1) * 512],
                                            start=(hi == 0),
                                            stop=(hi == HT - 1),
                                        )
                            for sg in range(2):
                                nc.scalar.activation(
                                    oT[:, j, sg * 1024:(sg + 1) * 1024],
                                    pss[sg].rearrange("p a b -> p (a b)"),
                                    AF.Identity,
                                    bias=b_sb[:, j:j + 1],
                                )
                    for st in range(ST):
                        ps = qkvps.tile([P, 2, 512], F32, tag="qkv")
                        for hi in range(HT):
                            lhsT = xT[:, hi, st * P:(st + 1) * P]
                            for dh in range(2):
                                nc.tensor.matmul(
                                    ps[:, dh, :],
                                    lhsT,
                                    wv_sb[:, hi, dh * 512:(dh + 1) * 512],
                                    start=(hi == 0),
                                    stop=(hi == HT - 1),
                                )
                        nc.vector.tensor_tensor(
                            v_sb[:, st, :],
                            ps.rearrange("p a b -> p (a b)"),
                            bv_bc[:, :],
                            ALU.add,
                        )

            # ---- Phase D: attention ----
            with (
                tc.tile_pool(name="pt", bufs=2) as ptpool,
                tc.tile_pool(name="dn", bufs=2) as dnpool,
                tc.tile_pool(name="rc", bufs=2) as rcpool,
                tc.tile_pool(name="sps", bufs=2, space="PSUM") as spool,
                tc.tile_pool(name="pvps", bufs=2, space="PSUM") as pvpool,
                tc.tile_pool(name="dnbps", bufs=2, space="PSUM") as dbpool,
            ):
                for h in range(NH):
                    for qb in range(SB):
                        qs = qT[:, h, qb * 512:(qb + 1) * 512]
                        PT = ptpool.tile([P, ST, 512], BF16, tag="pt")
                        for kg in range(8):
                            ps = spool.tile([P, 2, 512], F32, tag="sps")
                            for k2 in range(2):
                                kt = kg * 2 + k2
                                nc.tensor.matmul(
                                    ps[:, k2, :],
                                    kT[:, h, kt * P:(kt + 1) * P],
                                    qs,
                                    start=True,
                                    stop=True,
                                )
                            nc.scalar.activation(
                                PT[:, kg * 2:kg * 2 + 2, :], ps[:], AF.Exp,
                                scale=SCALE,
                            )
                        dn = dnpool.tile([P, 512], F32, tag="dn")
                        nc.vector.tensor_tensor(
                            dn, PT[:, 0, :], PT[:, 1, :], ALU.add
                        )
                        for kt in range(2, ST - 1):
                            nc.vector.tensor_tensor(dn, dn, PT[:, kt, :], ALU.add)
                        dnr = dnpool.tile([P, 512], BF16, tag="dnr")
                        nc.vector.tensor_tensor(
                            dnr, dn, PT[:, ST - 1, :], ALU.add
                        )
                        denb = dbpool.tile([P, 512], F32, tag="dnb")
                        nc.tensor.matmul(
                            denb, ones128, dnr, start=True, stop=True,
                        )
                        rcb = rcpool.tile([P, 512], F32, tag="rc")
                        nc.vector.reciprocal(rcb, denb)
                        pv = pvpool.tile([P, 512], F32, tag="pv")
                        for kt in range(ST):
                            nc.tensor.matmul(
                                pv,
                                v_sb[:, kt, h * DK:(h + 1) * DK],
                                PT[:, kt, :],
                                start=(kt == 0),
                                stop=(kt == ST - 1),
                            )
                        nc.vector.tensor_tensor(
                            an[:, h, qb * 512:(qb + 1) * 512],
                            pv,
                            rcb[:, :],
                            ALU.mult,
                        )

            # ---- Phase E: GELU + classify ----
            with tc.tile_pool(name="clps", bufs=2, space="PSUM") as clpool:
                for j in range(HT):
                    nc.scalar.activation(an[:, j, :], an[:, j, :], AF.Gelu)
                for qb in range(SB):
                    lps = clpool.tile([NC, 512], F32, tag="cl")
                    for j in range(HT):
                        nc.tensor.matmul(
                            lps,
                            wc_sb[:, j, :],
                            an[:, j, qb * 512:(qb + 1) * 512],
                            start=(j == 0),
                            stop=(j == HT - 1),
                        )
                    nc.scalar.activation(
                        louT[:, qb, :], lps, AF.Identity, bias=bc_sb
                    )
                nc.sync.dma_start(out[:, :], louT.rearrange("c a b -> c (a b)"))

    nc.finalize()
    return nc


def get_nc():
    if not _NC_CACHE:
        _NC_CACHE.append(_build())
    return _NC_CACHE[0]


def kernel(**inputs) -> np.ndarray:
    ins = {k: np.ascontiguousarray(np.asarray(v, dtype=np.float32))
           for k, v in inputs.items()}
    x = ins["x"]
    assert x.shape == (B, S, H), x.shape
    shared = {k: ins[k] for k in
              ("Wq", "bq", "Wk", "bk", "Wv", "bv", "Wc", "bc")}
    in_maps = [{"x": x[b], **shared} for b in range(B)]
    nc = get_nc()
    res = run_bass_kernel_spmd(nc, in_maps, core_ids=list(range(B)))
    outs = [np.asarray(res.results[b]["out"], dtype=np.float32).T
            for b in range(B)]
    return np.stack(outs, axis=0)


# revision 3
# speedup vs baseline: 1.3030x; 1.3030x over previous
"""TRN2 Bass kernel for nn_CustomHeadMultiHeadAttention (dense transformer).

Full inputs: x [8, 2048, 1024] f32 + QKV/classify weights. Sharding: pure
data parallelism — batch 8 across 8 NeuronCores, one batch element per core.
Each core runs the complete MHA + GELU + classify on its slice; no
collectives. Host only slices the batch and stacks/transposes the outputs.

Per-core pipeline (bf16 matmul operands, fp32 PSUM accumulation):
  xT    = PE-transpose(x)                  [h, s] layout
  v     = xT-chunks (lhsT) @ Wv + bv       [s, d] natural layout
  per head h (production overlaps the previous head's attention):
    qh/kh = Wq/Wk col-block (lhsT) @ xT + b    [d_k=128, s]
    per 512-wide q block:
      scores^T tiles = kh-chunk (lhsT) @ qh    [k=128, q=512]
      P^T  = exp(scores^T / sqrt(dk))          ACT PSUM->SBUF bf16
                                               (scores ~ N(0,1): no max-sub)
      dnr  = pairwise bf16 tree over P^T tiles (DVE 4x mode)
      denom broadcast = ones[128,128] @ dnr    one matmul -> [128, 512]
      attn^T = sum_kt v-chunk (lhsT) @ P^T[kt]
      an   = attn^T * reciprocal_approx_fast(denom)
  h^T   = gelu(an) in place; logits^T = Wc-chunks (lhsT) @ h^T + bc  [2, s]
Host transposes logits^T -> [2048, 2].
"""

import math
import sys

sys.path.insert(0, "/opt/trn_rl_repo")

import numpy as np

import concourse.bass as bass
import concourse.mybir as mybir
import concourse.tile as tile
from concourse import bacc
from concourse.bass_utils import run_bass_kernel_spmd
from concourse.masks import make_identity

AF = mybir.ActivationFunctionType
ALU = mybir.AluOpType
F32 = mybir.dt.float32
BF16 = mybir.dt.bfloat16

B = 8           # batch (== number of cores)
S = 2048        # sequence length
H = 1024        # hidden
NH = 8          # heads
DK = 128        # head dim
P = 128         # partitions
NC = 2          # classes
SB = S // 512   # 4 q/s blocks of 512
HT = H // P     # 8 hidden tiles
ST = S // P     # 16 seq tiles
SCALE = 1.0 / math.sqrt(DK)

_NC_CACHE = []


def _build():
    nc = bacc.Bacc(None, target_bir_lowering=False, debug=False)

    x = nc.dram_tensor("x", [S, H], F32, kind="ExternalInput")
    Wq = nc.dram_tensor("Wq", [H, H], F32, kind="ExternalInput")
    bq = nc.dram_tensor("bq", [H], F32, kind="ExternalInput")
    Wk = nc.dram_tensor("Wk", [H, H], F32, kind="ExternalInput")
    bk = nc.dram_tensor("bk", [H], F32, kind="ExternalInput")
    Wv = nc.dram_tensor("Wv", [H, H], F32, kind="ExternalInput")
    bv = nc.dram_tensor("bv", [H], F32, kind="ExternalInput")
    Wc = nc.dram_tensor("Wc", [H, NC], F32, kind="ExternalInput")
    bc = nc.dram_tensor("bc", [NC], F32, kind="ExternalInput")
    out = nc.dram_tensor("out", [NC, S], F32, kind="ExternalOutput")

    with tile.TileContext(nc) as tc:
        with (
            tc.tile_pool(name="persist", bufs=1) as persist,
            # all PSUM: one shared 2-bank-group pool + pv + denom-broadcast
            tc.tile_pool(name="g2ps", bufs=2, space="PSUM") as g2ps,
            tc.tile_pool(name="pvps", bufs=2, space="PSUM") as pvpool,
            tc.tile_pool(name="dnbps", bufs=2, space="PSUM") as dbpool,
        ):
            # --- constants / small params ---
            ident = persist.tile([P, P], BF16, tag="ident")
            make_identity(nc, ident)
            ones128 = persist.tile([P, P], BF16, tag="ones128")
            nc.vector.memset(ones128, 1.0)
            bq_sb = persist.tile([P, HT], F32, tag="bq")
            bk_sb = persist.tile([P, HT], F32, tag="bk")
            nc.sync.dma_start(bq_sb, bq.rearrange("(j p) -> p j", p=P))
            nc.sync.dma_start(bk_sb, bk.rearrange("(j p) -> p j", p=P))
            bv_bc = persist.tile([P, H], BF16, tag="bv")
            nc.gpsimd.dma_start(bv_bc, bv[None, :].to_broadcast((P, H)))
            bc_sb = persist.tile([NC, 1], F32, tag="bc")
            nc.sync.dma_start(bc_sb, bc[:, None])
            wc_sb = persist.tile([P, HT, NC], BF16, tag="wc")
            nc.gpsimd.dma_start(wc_sb, Wc.rearrange("(j p) c -> p j c", p=P))

            # --- big persistent tensors ---
            xT = persist.tile([P, HT, S], BF16, tag="xT")
            wv_sb = persist.tile([P, HT, H], BF16, tag="wv")
            nc.gpsimd.dma_start(wv_sb, Wv.rearrange("(o p) d -> p o d", p=P))
            wq_r = Wq.rearrange("(o p) d -> p o d", p=P)
            wk_r = Wk.rearrange("(o p) d -> p o d", p=P)
            v_sb = persist.tile([P, ST, H], BF16, tag="v")
            an = persist.tile([P, HT, S], BF16, tag="an")

            # --- load + PE-transpose x (f32 DRAM -> bf16 SBUF) ---
            with tc.tile_pool(name="xload", bufs=2) as xload:
                for st in range(ST):
                    xt = xload.tile([P, H], BF16, tag="xl")
                    nc.gpsimd.dma_start(xt, x[st * P:(st + 1) * P, :])
                    for jg in range(2):
                        ps = g2ps.tile([P, 4, P], BF16, tag="g2")
                        for j4 in range(4):
                            j = jg * 4 + j4
                            nc.tensor.transpose(
                                ps[:, j4, :], xt[:, j * P:(j + 1) * P], ident
                            )
                        nc.vector.tensor_copy(
                            xT[:, jg * 4:(jg + 1) * 4, st * P:(st + 1) * P],
                            ps[:],
                        )

            # --- V = x @ Wv + bv, natural [s, d] layout ---
            for st in range(ST):
                ps = g2ps.tile([P, 2, 512], F32, tag="g2")
                for hi in range(HT):
                    lhsT = xT[:, hi, st * P:(st + 1) * P]
                    for dh in range(2):
                        nc.tensor.matmul(
                            ps[:, dh, :],
                            lhsT,
                            wv_sb[:, hi, dh * 512:(dh + 1) * 512],
                            start=(hi == 0),
                            stop=(hi == HT - 1),
                        )
                nc.vector.tensor_tensor(
                    v_sb[:, st, :],
                    ps.rearrange("p a b -> p (a b)"),
                    bv_bc[:, :],
                    ALU.add,
                )

            # --- per-head: q/k projection then attention ---
            with (
                tc.tile_pool(name="wj", bufs=3) as wjpool,
                tc.tile_pool(name="qk", bufs=2) as qkpool,
                tc.tile_pool(name="pt", bufs=2) as ptpool,
                tc.tile_pool(name="tadd", bufs=1) as tapool,
                tc.tile_pool(name="rc", bufs=2) as rcpool,
            ):
                for h in range(NH):
                    qh = qkpool.tile([P, S], BF16, tag="qh")
                    kh = qkpool.tile([P, S], BF16, tag="kh")
                    for w_r, b_sb, oT in ((wq_r, bq_sb, qh), (wk_r, bk_sb, kh)):
                        wj = wjpool.tile([P, HT, P], BF16, tag="wj")
                        nc.gpsimd.dma_start(wj, w_r[:, :, h * P:(h + 1) * P])
                        for sg in range(2):
                            ps = g2ps.tile([P, 2, 512], F32, tag="g2")
                            for hi in range(HT):
                                for sh in range(2):
                                    nc.tensor.matmul(
                                        ps[:, sh, :],
                                        wj[:, hi, :],
                                        xT[:, hi,
                                           sg * 1024 + sh * 512:
                                           sg * 1024 + (sh + 1) * 512],
                                        start=(hi == 0),
                                        stop=(hi == HT - 1),
                                    )
                            nc.vector.tensor_tensor(
                                oT[:, sg * 1024:(sg + 1) * 1024],
                                ps.rearrange("p a b -> p (a b)"),
                                b_sb[:, h:h + 1].to_broadcast((P, 1024)),
                                ALU.add,
                            )

                    for qb in range(SB):
                        qs = qh[:, qb * 512:(qb + 1) * 512]
                        PT = ptpool.tile([P, ST, 512], BF16, tag="pt")
                        for kg in range(8):
                            ps = g2ps.tile([P, 2, 512], F32, tag="g2")
                            for k2 in range(2):
                                kt = kg * 2 + k2
                                nc.tensor.matmul(
                                    ps[:, k2, :],
                                    kh[:, kt * P:(kt + 1) * P],
                                    qs,
                                    start=True,
                                    stop=True,
                                )
                            nc.scalar.activation(
                                PT[:, kg * 2:kg * 2 + 2, :], ps[:], AF.Exp,
                                scale=SCALE,
                            )
                        # denominator: pairwise bf16 tree (DVE 4x mode)
                        tt = tapool.tile([P, 8, 512], BF16, tag="tt")
                        for i in range(8):
                            nc.vector.tensor_tensor(
                                tt[:, i, :], PT[:, 2 * i, :], PT[:, 2 * i + 1, :],
                                ALU.add,
                            )
                        for i in range(4):
                            nc.vector.tensor_tensor(
                                tt[:, i, :], tt[:, 2 * i, :], tt[:, 2 * i + 1, :],
                                ALU.add,
                            )
                        nc.vector.tensor_tensor(
                            tt[:, 0, :], tt[:, 0, :], tt[:, 1, :], ALU.add
                        )
                        nc.vector.tensor_tensor(
                            tt[:, 1, :], tt[:, 2, :], tt[:, 3, :], ALU.add
                        )
                        nc.vector.tensor_tensor(
                            tt[:, 0, :], tt[:, 0, :], tt[:, 1, :], ALU.add
                        )
                        denb = dbpool.tile([P, 512], F32, tag="dnb")
                        nc.tensor.matmul(
                            denb, ones128, tt[:, 0, :], start=True, stop=True,
                        )
                        rcb = rcpool.tile([P, 512], F32, tag="rc")
                        nc.vector.reciprocal_approx_fast(rcb, denb)
                        pv = pvpool.tile([P, 512], F32, tag="pv")
                        for kt in range(ST):
                            nc.tensor.matmul(
                                pv,
                                v_sb[:, kt, h * DK:(h + 1) * DK],
                                PT[:, kt, :],
                                start=(kt == 0),
                                stop=(kt == ST - 1),
                            )
                        nc.vector.tensor_tensor(
                            an[:, h, qb * 512:(qb + 1) * 512],
                            pv,
                            rcb[:, :],
                            ALU.mult,
                        )

            # --- GELU + classify ---
            with tc.tile_pool(name="lou", bufs=4) as loupool:
                for j in range(HT):
                    nc.scalar.activation(an[:, j, :], an[:, j, :], AF.Gelu)
                for qb in range(SB):
                    lps = pvpool.tile([NC, 512], F32, tag="pv")
                    for j in range(HT):
                        nc.tensor.matmul(
                            lps,
                            wc_sb[:, j, :],
                            an[:, j, qb * 512:(qb + 1) * 512],
                            start=(j == 0),
                            stop=(j == HT - 1),
                        )
                    lo = loupool.tile([NC, 512], F32, tag="lou")
                    nc.vector.tensor_tensor(
                        lo, lps, bc_sb.to_broadcast((NC, 512)), ALU.add
                    )
                    nc.sync.dma_start(out[:, qb * 512:(qb + 1) * 512], lo)

    nc.finalize()
    return nc


def get_nc():
    if not _NC_CACHE:
        _NC_CACHE.append(_build())
    return _NC_CACHE[0]


def kernel(**inputs) -> np.ndarray:
    ins = {k: np.ascontiguousarray(np.asarray(v, dtype=np.float32))
           for k, v in inputs.items()}
    x = ins["x"]
    assert x.shape == (B, S, H), x.shape
    shared = {k: ins[k] for k in
              ("Wq", "bq", "Wk", "bk", "Wv", "bv", "Wc", "bc")}
    in_maps = [{"x": x[b], **shared} for b in range(B)]
    nc = get_nc()
    res = run_bass_kernel_spmd(nc, in_maps, core_ids=list(range(B)))
    outs = [np.asarray(res.results[b]["out"], dtype=np.float32).T
            for b in range(B)]
    return np.stack(outs, axis=0)
